# revision 1
# baseline (speedup 1.0000x reference)
"""Cross-attention Trainium2 kernel (8 NeuronCores).

Sharding: batch (2) x head-groups (4 groups of 4 heads) = 8 shards.
Each core computes q/k/v projections for its 4 heads (256 cols of
Wq/Wk/Wv), attention for those heads, and a partial out-projection
through its 256 rows of Wo.  The host sums the 4 partial outputs per
batch (the reduction of the head-parallel out_proj) and adds the
bv @ Wo + bo correction, which commutes exactly through the softmax
average.

Layout strategy on-core:
  - x/ctx are PE-transposed (xT: [d, s]) so projections contract d on
    partitions; projections emit qT/kT ([head_dim, s]) and v (natural).
  - scores are computed transposed (ST = k @ qT -> [sk, sq]) so the
    exp'd tiles feed the attention matmul directly as the stationary
    operand; a ones-column in v gives the softmax denominator for free.
  - matmuls run as float32r (full-rate fp32, operands typed f32r
    end-to-end to satisfy walrus rounding rules); softmax P, v, attnT
    and Wo are fp16 (the fp32r ISA check forbids PSUM dst partition
    base 64, which the odd heads' attn transpose needs).
  - emission is software-pipelined: projections interleave with the
    first two windows' scores, each window's attn matmuls follow the
    next window's scores+exp, out_proj weaves into late windows.
"""

import numpy as np

import concourse.bass as bass
import concourse.mybir as mybir
import concourse.tile as tile
from concourse import bacc

B, SQ, SK, D, H, HS = 2, 2048, 2048, 1024, 16, 64
SCALE = HS ** -0.5
NCORES = 8
HG = 4            # heads per core
DG = HG * HS      # 256 projection cols per core

F32 = mybir.dt.float32
F32R = mybir.dt.float32r
F16 = mybir.dt.float16


def build_program(fast_mm: bool = True, pipeline: bool = True, loop_iters: int = 0):
    """Build the per-core SPMD Bass program."""
    MMDT = F32R if fast_mm else F32

    nc = bacc.Bacc(None, target_bir_lowering=False, debug=False,
                   num_devices=NCORES)
    x_d = nc.dram_tensor("x", [SQ, D], F16, kind="ExternalInput")
    c_d = nc.dram_tensor("ctx", [SK, D], F16, kind="ExternalInput")
    wq_d = nc.dram_tensor("wq", [D, DG], F16, kind="ExternalInput")
    wk_d = nc.dram_tensor("wk", [D, DG], F16, kind="ExternalInput")
    wv_d = nc.dram_tensor("wv", [D, DG], F16, kind="ExternalInput")
    wo_d = nc.dram_tensor("wo", [DG, D], F16, kind="ExternalInput")
    bq_d = nc.dram_tensor("bq", [DG], F32, kind="ExternalInput")
    bk_d = nc.dram_tensor("bk", [DG], F32, kind="ExternalInput")
    i_d = nc.dram_tensor("ident", [128, 128], MMDT, kind="ExternalInput")
    i16_d = nc.dram_tensor("ident16", [128, 128], F16, kind="ExternalInput")
    out_d = nc.dram_tensor("out", [SQ, D], F32, kind="ExternalOutput")

    with tile.TileContext(nc) as tc:
        with (
            tc.tile_pool(name="const", bufs=1) as cp,
            tc.tile_pool(name="persist", bufs=1) as psb,
            tc.tile_pool(name="xw", bufs=8) as xwp,
            tc.tile_pool(name="xtw", bufs=2) as xtwp,
            tc.tile_pool(name="expp", bufs=28) as ep,
            tc.tile_pool(name="fin", bufs=4) as fpool,
            tc.tile_pool(name="outp", bufs=3) as opool,
            tc.tile_pool(name="pp", bufs=2, space="PSUM") as pp,
            tc.tile_pool(name="stp", bufs=2, space="PSUM") as stp,
            tc.tile_pool(name="atp", bufs=2, space="PSUM") as atp,
        ):
            import contextlib
            loop_ctx = tc.For_i(0, loop_iters, 1) if loop_iters else contextlib.nullcontext()
            loop_ctx.__enter__()
            ident = cp.tile([128, 128], MMDT)
            nc.sync.dma_start(out=ident, in_=i_d[:])
            ident16 = cp.tile([128, 128], F16, tag="ident16")
            nc.sync.dma_start(out=ident16, in_=i16_d[:])

            wq_sb = cp.tile([128, 8, DG], F16, tag="wq")
            wk_sb = cp.tile([128, 8, DG], F16, tag="wk")
            wv_sb = cp.tile([128, 8, DG], F16, tag="wv")
            wo_sb = cp.tile([128, 2, D], F16, tag="wo")
            bq_sb = cp.tile([128, 2], F32, tag="bq")
            bk_sb = cp.tile([128, 2], F32, tag="bk")
            def load_weights_qx():
                nc.sync.dma_start(out=wq_sb, in_=wq_d[:].rearrange("(c p) n -> p c n", p=128))
                nc.sync.dma_start(out=bq_sb, in_=bq_d[:].rearrange("(c p) -> p c", p=128))

            def load_weights_kv():
                nc.sync.dma_start(out=wk_sb, in_=wk_d[:].rearrange("(c p) n -> p c n", p=128))
                nc.sync.dma_start(out=wv_sb, in_=wv_d[:].rearrange("(c p) n -> p c n", p=128))
                nc.sync.dma_start(out=bk_sb, in_=bk_d[:].rearrange("(c p) -> p c", p=128))

            def load_weights_o():
                nc.sync.dma_start(out=wo_sb, in_=wo_d[:].rearrange("(c p) n -> p c n", p=128))

            # persistent activations, split per producing window so the
            # scheduler's dependencies stay fine-grained
            qTs = [psb.tile([128, 2, 512], F16, tag=f"qT{w}", name=f"qT{w}") for w in range(4)]
            kTs = [psb.tile([128, 2, 512], F16, tag=f"kT{w}", name=f"kT{w}") for w in range(4)]
            vAs = [psb.tile([128, 4, HG, 68], F16, tag=f"vA{w}", name=f"vA{w}") for w in range(4)]
            aTs = [psb.tile([128, 2, 128], F16, tag=f"aT{s}", name=f"aT{s}") for s in range(16)]

            for w in range(4):
                nc.vector.memset(vAs[w][:], 1.0)

            def proj_window(src_d, dst_T, bias_sb, w_sb, with_v, w, after_dma=None):
                xts = []
                for i in range(4):
                    xt = xwp.tile([128, D], F16, tag="xw")
                    r0 = (w * 4 + i) * 128
                    nc.sync.dma_start(out=xt, in_=src_d[r0:r0 + 128, :])
                    xts.append(xt)
                if after_dma is not None:
                    after_dma()
                xtw = xtwp.tile([128, 8, 512], F16, tag="xtw")
                for dc in range(8):
                    pt = pp.tile([128, 512], F16, tag="pp")
                    for i in range(4):
                        nc.tensor.transpose(
                            (pt[:, i * 128:(i + 1) * 128]),
                            (xts[i][:, dc * 128:(dc + 1) * 128]),
                            (ident16),
                        )
                    nc.vector.tensor_copy(xtw[:, dc, :], pt)
                for c in range(2):
                    pq = pp.tile([128, 512], F32, tag="pp")
                    for dc in range(8):
                        nc.tensor.matmul(
                            pq,
                            (w_sb[:, dc, c * 128:(c + 1) * 128]),
                            (xtw[:, dc, :]),
                            start=(dc == 0), stop=(dc == 7),
                        )
                    nc.vector.tensor_scalar_add(
                        dst_T[w][:, c, :], pq, bias_sb[:, c:c + 1])
                if with_v:
                    for s in range(4):
                        # attention psum pool is idle during projections
                        pv = atp.tile([128, 512], F32, tag="at")
                        for dc in range(8):
                            nc.tensor.matmul(
                                pv[:, :DG],
                                (xtw[:, dc, s * 128:(s + 1) * 128]),
                                (wv_sb[:, dc, :]),
                                start=(dc == 0), stop=(dc == 7),
                            )
                        nc.vector.tensor_copy(
                            vAs[w][:, s, :, 0:64],
                            pv[:, :DG].rearrange("p (h e) -> p h e", e=64),
                        )

            def proj_x(w, after_dma=None):
                proj_window(x_d, qTs, bq_sb, wq_sb, False, w, after_dma)

            def proj_ctx(w, after_dma=None):
                proj_window(c_d, kTs, bk_sb, wk_sb, True, w, after_dma)

            # attention per head / sq-window of 1024, software-pipelined:
            # window w's attn-matmuls are emitted after window w+1's
            # scores+exp so ACT (exp) is never starved.
            def emit_scores_exp(h, sqw, skcs):
                p0 = 64 * (h % 2)
                t = h // 2
                exs = []
                for skc in skcs:
                    st = stp.tile([128, 1024], F32, tag="st")
                    for half in range(2):
                        qw = sqw * 2 + half
                        nc.tensor.matmul(
                            st[:, half * 512:(half + 1) * 512],
                            (kTs[skc // 4][p0:p0 + 64, t,
                                             (skc % 4) * 128:(skc % 4 + 1) * 128]),
                            (qTs[qw][p0:p0 + 64, t, :]),
                            start=True, stop=True,
                        )
                    ex = ep.tile([128, 1024], F16, tag="ex")
                    nc.scalar.activation(
                        ex, st, mybir.ActivationFunctionType.Exp,
                        scale=SCALE)
                    exs.append(ex)
                return exs

            def emit_attnv_fin(h, sqw, exs, per_j=None):
                p0 = 64 * (h % 2)
                t = h // 2
                # attn accumulation: one psum bank per sq-chunk j
                for j in range(8):
                    at = atp.tile([128, 512], F32, tag="at")
                    for skc in range(16):
                        nc.tensor.matmul(
                            at[:, 0:68],
                            exs[skc][:, j * 128:(j + 1) * 128],
                            vAs[skc // 4][:, skc % 4, h, :],
                            start=(skc == 0), stop=(skc == 15),
                        )
                    # normalize + transpose into aT
                    rc = fpool.tile([128, 1], F32, tag="rc")
                    nc.vector.reciprocal(rc, at[:, 64:65])
                    ad = fpool.tile([128, 64], F16, tag="ad")
                    nc.vector.tensor_scalar_mul(ad, at[:, 0:64], rc)
                    pt2 = pp.tile([128, 128], F16, tag="pp")
                    nc.tensor.transpose(pt2[p0:p0 + 64, :], ad, ident16)
                    nc.vector.tensor_copy(
                        aTs[sqw * 8 + j][p0:p0 + 64, t, :],
                        pt2[p0:p0 + 64, :])
                    if per_j is not None:
                        per_j(j)

            # out projection for a range of sq chunks (partial out: this
            # core's 256 attn cols)
            def emit_out_proj(sqcs):
                for sqc in sqcs:
                    ot = opool.tile([128, D], F32, tag="ot")
                    for n2 in range(2):
                        po = pp.tile([128, 512], F32, tag="pp")
                        for kc in range(2):
                            nc.tensor.matmul(
                                po,
                                (aTs[sqc][:, kc, :]),
                                (wo_sb[:, kc, n2 * 512:(n2 + 1) * 512]),
                                start=(kc == 0), stop=(kc == 1),
                            )
                        nc.vector.tensor_copy(ot[:, n2 * 512:(n2 + 1) * 512], po)
                    nc.sync.dma_start(
                        out=out_d[sqc * 128:(sqc + 1) * 128, :], in_=ot)

            if pipeline:
                # interleave projections with the first TWO attention
                # windows' scores so ACT (exp) starts as early as possible
                # (window 1 skc 8-15 depend on late ctx windows; window 2's
                # early skc only need ctx windows 0-1 and fill those gaps)
                proj_x(0, after_dma=load_weights_qx)
                proj_x(1)
                proj_ctx(0, after_dma=load_weights_kv)
                e1 = emit_scores_exp(0, 0, range(0, 4))
                proj_ctx(1)
                e1 += emit_scores_exp(0, 0, range(4, 8))
                e2 = emit_scores_exp(1, 0, range(0, 4))
                proj_x(2)
                proj_ctx(2)
                e1 += emit_scores_exp(0, 0, range(8, 12))
                e2 += emit_scores_exp(1, 0, range(4, 8))
                proj_x(3)
                proj_ctx(3)
                load_weights_o()
                e1 += emit_scores_exp(0, 0, range(12, 16))
                e2 += emit_scores_exp(1, 0, range(8, 12))
                emit_attnv_fin(0, 0, e1)
                e2 += emit_scores_exp(1, 0, range(12, 16))
                pending = (1, 0, e2)
                # out_proj chunks woven into the later (ACT-bound) windows
                op_after = {2: range(0, 2), 3: range(2, 4), 4: range(4, 6),
                            5: range(6, 8)}
                windows = [(h, sqw) for sqw in range(2) for h in range(HG)]
                for i, (h, sqw) in enumerate(windows[2:]):
                    exs = emit_scores_exp(h, sqw, range(0, 4))
                    emit_attnv_fin(*pending)
                    if i in op_after:
                        emit_out_proj(op_after[i])
                    pending = (h, sqw, exs)
                    exs += emit_scores_exp(h, sqw, range(4, 16))
                # final window: out_proj chunk 8+j right after its fin(j)
                emit_attnv_fin(*pending,
                               per_j=lambda j: emit_out_proj([8 + j]))
            else:
                load_weights_qx()
                load_weights_kv()
                load_weights_o()
                for w in range(4):
                    proj_x(w)
                for w in range(4):
                    proj_ctx(w)
                for h in range(HG):
                    for sqw in range(2):
                        exs = emit_scores_exp(h, sqw, range(16))
                        emit_attnv_fin(h, sqw, exs)
                emit_out_proj(range(16))
            loop_ctx.__exit__(None, None, None)

    nc.compile()
    return nc


_NC = None


def _program():
    global _NC
    if _NC is None:
        _NC = build_program()
    return _NC


def _f32(a):
    return np.ascontiguousarray(np.asarray(a, dtype=np.float32))


def kernel(inputs, context, Wq, bq, Wk, bk, Wv, bv, Wo, bo):
    from concourse.bass_utils import run_bass_kernel_spmd

    inputs = _f32(inputs)
    context = _f32(context)
    Wq, bq, Wk, bk = _f32(Wq), _f32(bq), _f32(Wk), _f32(bk)
    Wv, bv, Wo, bo = _f32(Wv), _f32(bv), _f32(Wo), _f32(bo)

    nc = _program()
    in_maps = []
    for core in range(NCORES):
        b, g = core // HG, core % HG
        sl = slice(DG * g, DG * (g + 1))
        in_maps.append({
            "x": np.ascontiguousarray(inputs[b].astype(np.float16)),
            "ctx": np.ascontiguousarray(context[b].astype(np.float16)),
            "wq": np.ascontiguousarray(Wq[:, sl].astype(np.float16)),
            "wk": np.ascontiguousarray(Wk[:, sl].astype(np.float16)),
            "wv": np.ascontiguousarray(Wv[:, sl].astype(np.float16)),
            "wo": np.ascontiguousarray(Wo[sl, :].astype(np.float16)),
            "bq": _f32(bq[sl]),
            "bk": _f32(bk[sl]),
            "ident": np.eye(128, dtype=np.float32),
            "ident16": np.eye(128, dtype=np.float16),
        })
    res = run_bass_kernel_spmd(nc, in_maps, list(range(NCORES)))
    outs = [res.results[i]["out"] for i in range(NCORES)]
    corr = (bv.astype(np.float64) @ Wo.astype(np.float64)
            + bo.astype(np.float64)).astype(np.float32)
    full = np.stack([
        outs[0] + outs[1] + outs[2] + outs[3],
        outs[4] + outs[5] + outs[6] + outs[7],
    ]) + corr
    return full.astype(np.float32)



# revision 10
# speedup vs baseline: 1.4577x; 1.4577x over previous
"""Cross-attention Trainium2 kernel (8 NeuronCores).

Sharding: batch (2) x head-groups (4 groups of 4 heads) = 8 shards.
Each core computes q/k/v projections for its 4 heads (256 cols of
Wq/Wk/Wv), attention for those heads, and a partial out-projection
through its 256 rows of Wo.  The host sums the 4 partial outputs per
batch and adds the bv @ Wo + bo correction, which commutes exactly
through the softmax average.

Layout strategy on-core (v2):
  - x/ctx are transposed on the HOST (xT: [d, s]) so projections
    contract d on partitions with no on-chip transposes; projections
    emit qT/kT ([dims, s], stationary W-chunks) and v natural
    (stationary xT-chunks).
  - scores are computed transposed (ST = k @ qT -> [sk, sq]) so the
    exp'd tiles feed the attention matmul directly as the stationary
    operand; ones-columns appended to v give the softmax denominator
    for free.
  - attention accumulates 4 sq-chunks into one PSUM bank so the
    softmax normalization runs as one reciprocal + one broadcast
    multiply per 4 chunks.
  - the emission order is tuned so the ACT engine (exp: the pacing
    engine at ~133us of work) starts as early as possible and is
    never starved: score batches are h-major and projection / attn /
    out-proj work is woven between them at the ACT drain rate.
"""

import numpy as np

import concourse.bass as bass
import concourse.mybir as mybir
import concourse.tile as tile
from concourse import bacc

B, SQ, SK, D, H, HS = 2, 2048, 2048, 1024, 16, 64
SCALE = HS ** -0.5
NCORES = 8
HG = 4            # heads per core
DG = HG * HS      # 256 projection cols per core

F32 = mybir.dt.float32
F16 = mybir.dt.float16


def build_program(loop_iters: int = 0):
    """Build the per-core SPMD Bass program."""
    nc = bacc.Bacc(None, target_bir_lowering=False, debug=False,
                   num_devices=NCORES)
    xT_d = nc.dram_tensor("xT", [D, SQ], F16, kind="ExternalInput")
    cT_d = nc.dram_tensor("cT", [D, SK], F16, kind="ExternalInput")
    wq_d = nc.dram_tensor("wq", [D, DG], F16, kind="ExternalInput")
    wk_d = nc.dram_tensor("wk", [D, DG], F16, kind="ExternalInput")
    wv_d = nc.dram_tensor("wv", [D, DG], F16, kind="ExternalInput")
    wo_d = nc.dram_tensor("wo", [DG, D], F16, kind="ExternalInput")
    bq_d = nc.dram_tensor("bq", [DG], F32, kind="ExternalInput")
    bk_d = nc.dram_tensor("bk", [DG], F32, kind="ExternalInput")
    i16_d = nc.dram_tensor("ident16", [128, 128], F16, kind="ExternalInput")
    out_d = nc.dram_tensor("out", [SQ, D], F16, kind="ExternalOutput")

    with tile.TileContext(nc) as tc:
        with (
            tc.tile_pool(name="const", bufs=1) as cp,
            tc.tile_pool(name="persist", bufs=1) as psb,
            tc.tile_pool(name="xw", bufs=4) as xwp,
            tc.tile_pool(name="expp", bufs=52) as ep,
            tc.tile_pool(name="fin", bufs=4) as fpool,
            tc.tile_pool(name="outp", bufs=3) as opool,
            tc.tile_pool(name="pp", bufs=2, space="PSUM") as pp,
            tc.tile_pool(name="stp", bufs=2, space="PSUM") as stp,
            tc.tile_pool(name="atp", bufs=2, space="PSUM") as atp,
        ):
            import contextlib
            loop_ctx = tc.For_i(0, loop_iters, 1) if loop_iters else contextlib.nullcontext()
            loop_ctx.__enter__()
            ident16 = cp.tile([128, 128], F16, tag="ident16")
            wq_sb = cp.tile([128, 8, DG], F16, tag="wq")
            wk_sb = cp.tile([128, 8, DG], F16, tag="wk")
            wv_sb = cp.tile([128, 8, DG], F16, tag="wv")
            wo_sb = cp.tile([128, 2, D], F16, tag="wo")
            bq_sb = cp.tile([128, 2], F32, tag="bq")
            bk_sb = cp.tile([128, 2], F32, tag="bk")

            # two DMA queues: x/q-side on SP, ctx-side on the (otherwise
            # idle) GPSIMD queue, so the serial DMA lead-in halves.
            def dma_wq():
                nc.sync.dma_start(out=wq_sb, in_=wq_d[:].rearrange("(c p) n -> p c n", p=128))

            def dma_bqk():
                nc.sync.dma_start(out=bq_sb, in_=bq_d[:].rearrange("(c p) -> p c", p=128))
                nc.sync.dma_start(out=bk_sb, in_=bk_d[:].rearrange("(c p) -> p c", p=128))

            def dma_wk():
                nc.gpsimd.dma_start(out=ident16, in_=i16_d[:])
                nc.gpsimd.dma_start(out=wk_sb, in_=wk_d[:].rearrange("(c p) n -> p c n", p=128))

            def dma_wv():
                nc.gpsimd.dma_start(out=wv_sb, in_=wv_d[:].rearrange("(c p) n -> p c n", p=128))

            def dma_wo():
                nc.gpsimd.dma_start(out=wo_sb, in_=wo_d[:].rearrange("(c p) n -> p c n", p=128))

            xws = [None] * 4
            cws = [None] * 4

            def dma_x(w):
                xws[w] = xwp.tile([128, 8, 512], F16, tag="xw", name=f"xw{w}")
                nc.sync.dma_start(
                    out=xws[w],
                    in_=xT_d[:, w * 512:(w + 1) * 512].rearrange(
                        "(c p) s -> p c s", p=128))

            def dma_c(w):
                cws[w] = xwp.tile([128, 8, 512], F16, tag="xw", name=f"cw{w}")
                nc.gpsimd.dma_start(
                    out=cws[w],
                    in_=cT_d[:, w * 512:(w + 1) * 512].rearrange(
                        "(c p) s -> p c s", p=128))

            # persistent activations
            qTs = [psb.tile([128, 2, 512], F16, tag=f"qT{w}", name=f"qT{w}") for w in range(4)]
            kTs = [psb.tile([128, 2, 512], F16, tag=f"kT{w}", name=f"kT{w}") for w in range(4)]
            vAs = [psb.tile([128, 4, HG, 68], F16, tag=f"vA{w}", name=f"vA{w}") for w in range(4)]
            aTw = [psb.tile([128, 2, 1024], F16, tag=f"aT{s}", name=f"aT{s}") for s in range(2)]

            for w in range(4):
                nc.vector.memset(vAs[w][:, :, :, 64:68], 1.0)

            def proj_qk_c(src, dst_T, bias_sb, w_sb, w, c):
                # one 128-dim chunk of qT/kT window w (c=0: heads 0/1)
                pq = pp.tile([128, 512], F32, tag="pp")
                for dc in range(8):
                    nc.tensor.matmul(
                        pq,
                        (w_sb[:, dc, c * 128:(c + 1) * 128]),
                        (src[:, dc, :]),
                        start=(dc == 0), stop=(dc == 7),
                    )
                nc.vector.tensor_scalar_add(
                    dst_T[w][:, c, :], pq, bias_sb[:, c:c + 1])

            def proj_q(w, c):
                proj_qk_c(xws[w], qTs, bq_sb, wq_sb, w, c)

            def proj_k(w, c):
                proj_qk_c(cws[w], kTs, bk_sb, wk_sb, w, c)

            def proj_v(w, s):
                # one 128-row sk chunk of v (natural layout, stationary ctxT)
                pv = atp.tile([128, 512], F32, tag="at")
                for dc in range(8):
                    nc.tensor.matmul(
                        pv[:, :DG],
                        (cws[w][:, dc, s * 128:(s + 1) * 128]),
                        (wv_sb[:, dc, :]),
                        start=(dc == 0), stop=(dc == 7),
                    )
                nc.vector.tensor_copy(
                    vAs[w][:, s, :, 0:64],
                    pv[:, :DG].rearrange("p (h e) -> p h e", e=64),
                )

            # score+exp chunk: ST[sk 128, sq 1024] for (h, sqw, skc)
            exd = {}

            def emit_st(h, sqw, skcs):
                p0 = 64 * (h % 2)
                t = h // 2
                for skc in skcs:
                    st = stp.tile([128, 1024], F32, tag="st")
                    for half in range(2):
                        nc.tensor.matmul(
                            st[:, half * 512:(half + 1) * 512],
                            (kTs[skc // 4][p0:p0 + 64, t,
                                           (skc % 4) * 128:(skc % 4 + 1) * 128]),
                            (qTs[sqw * 2 + half][p0:p0 + 64, t, :]),
                            start=True, stop=True,
                        )
                    ex = ep.tile([128, 1024], F16, tag="ex")
                    nc.scalar.activation(
                        ex, st, mybir.ActivationFunctionType.Exp,
                        scale=SCALE)
                    exd[(h, sqw, skc)] = ex

            # attention window (h, sqw): two groups of 4 sq-chunks; each
            # group accumulates in one PSUM bank -> batched normalize.
            def emit_attn(h, sqw, after_group=None):
                p0 = 64 * (h % 2)
                t = h // 2
                for jj in range(2):
                    at4 = atp.tile([128, 512], F32, tag="at")
                    for j4 in range(4):
                        jc = j4  # sq-chunk within the exp tile half
                        for skc in range(16):
                            nc.tensor.matmul(
                                at4[:, j4 * 128:j4 * 128 + 68],
                                exd[(h, sqw, skc)][:, (jj * 4 + j4) * 128:
                                                   (jj * 4 + j4 + 1) * 128],
                                vAs[skc // 4][:, skc % 4, h, :],
                                start=(skc == 0), stop=(skc == 15),
                            )
                    # batched normalize: 1 reciprocal + 1 bcast multiply
                    atv = at4[:].rearrange("p (j c) -> p j c", c=128)
                    rc4 = fpool.tile([128, 4], F32, tag="rc")
                    nc.vector.reciprocal(rc4[:, :].unsqueeze(2), atv[:, :, 64:65])
                    ad4 = fpool.tile([128, 4, 64], F16, tag="ad")
                    nc.vector.tensor_tensor(
                        out=ad4[:, :, :],
                        in0=atv[:, :, 0:64],
                        in1=rc4[:, :].unsqueeze(2).broadcast_to((128, 4, 64)),
                        op=mybir.AluOpType.mult,
                    )
                    ptw = atp.tile([128, 512], F16, tag="at")
                    for j4 in range(4):
                        nc.tensor.transpose(
                            ptw[p0:p0 + 64, j4 * 128:(j4 + 1) * 128],
                            ad4[:, j4, :],
                            ident16,
                        )
                    nc.vector.tensor_copy(
                        aTw[sqw][p0:p0 + 64, t, jj * 512:(jj + 1) * 512],
                        ptw[p0:p0 + 64, :])
                    if after_group is not None:
                        after_group(jj)

            # out projection for a range of sq chunks (partial out: this
            # core's 256 attn cols)
            def emit_out_proj(sqcs):
                for sqc in sqcs:
                    sqw, i = sqc // 8, sqc % 8
                    ot = opool.tile([128, D], F16, tag="ot")
                    for n2 in range(2):
                        po = pp.tile([128, 512], F32, tag="pp")
                        for kc in range(2):
                            nc.tensor.matmul(
                                po,
                                (aTw[sqw][:, kc, i * 128:(i + 1) * 128]),
                                (wo_sb[:, kc, n2 * 512:(n2 + 1) * 512]),
                                start=(kc == 0), stop=(kc == 1),
                            )
                        nc.vector.tensor_copy(ot[:, n2 * 512:(n2 + 1) * 512], po)
                    nc.sync.dma_start(
                        out=out_d[sqc * 128:(sqc + 1) * 128, :], in_=ot)

            # ---------------- emission schedule ----------------
            # DMAs: q-side first (scores need qT0+qT1+kT0 first), ctx
            # windows next (attn needs all v windows before first attn),
            # x2/x3 last (only needed for sqw=1 scores, ~90us in).
            dma_w()
            dma_x(0)
            dma_x(1)
            dma_wk()
            dma_c(0)
            dma_wv()
            dma_c(1)
            dma_c(2)
            dma_c(3)
            dma_wo()
            dma_x(2)
            dma_x(3)

            proj_q(0)
            proj_q(1)
            proj_k(0)
            emit_st(0, 0, range(0, 4))
            proj_k(1)
            emit_st(0, 0, range(4, 8))
            proj_v(0)
            proj_k(2)
            emit_st(0, 0, range(8, 12))
            proj_v(1)
            proj_k(3)
            emit_st(0, 0, range(12, 16))
            proj_v(2)
            emit_st(1, 0, range(0, 8))
            proj_v(3)
            emit_st(1, 0, range(8, 16))
            proj_q(2)
            proj_q(3)
            emit_st(2, 0, range(0, 8))
            emit_attn(0, 0)
            emit_st(2, 0, range(8, 16))
            emit_st(3, 0, range(0, 4))
            emit_attn(1, 0)
            emit_st(3, 0, range(4, 16))
            emit_st(0, 1, range(0, 4))
            emit_attn(2, 0)
            emit_st(0, 1, range(4, 16))
            emit_attn(3, 0)
            emit_st(1, 1, range(0, 8))
            emit_out_proj(range(0, 2))
            emit_st(1, 1, range(8, 16))
            emit_out_proj(range(2, 4))
            emit_attn(0, 1)
            emit_st(2, 1, range(0, 8))
            emit_out_proj(range(4, 6))
            emit_st(2, 1, range(8, 16))
            emit_attn(1, 1)
            emit_st(3, 1, range(0, 8))
            emit_out_proj(range(6, 8))
            emit_st(3, 1, range(8, 16))
            emit_attn(2, 1)
            emit_attn(3, 1,
                      after_group=lambda jj: emit_out_proj(
                          range(8 + jj * 4, 12 + jj * 4)))
            loop_ctx.__exit__(None, None, None)

    nc.compile()
    return nc


_NC = None


def _program():
    global _NC
    if _NC is None:
        _NC = build_program()
    return _NC


def _f32(a):
    return np.ascontiguousarray(np.asarray(a, dtype=np.float32))


def make_in_maps(inputs, context, Wq, bq, Wk, bk, Wv, bv, Wo, bo):
    xT = [np.ascontiguousarray(np.asarray(inputs[b]).T.astype(np.float16))
          for b in range(B)]
    cT = [np.ascontiguousarray(np.asarray(context[b]).T.astype(np.float16))
          for b in range(B)]
    in_maps = []
    for core in range(NCORES):
        b, g = core // HG, core % HG
        sl = slice(DG * g, DG * (g + 1))
        in_maps.append({
            "xT": xT[b],
            "cT": cT[b],
            "wq": np.ascontiguousarray(np.asarray(Wq)[:, sl].astype(np.float16)),
            "wk": np.ascontiguousarray(np.asarray(Wk)[:, sl].astype(np.float16)),
            "wv": np.ascontiguousarray(np.asarray(Wv)[:, sl].astype(np.float16)),
            "wo": np.ascontiguousarray(np.asarray(Wo)[sl, :].astype(np.float16)),
            "bq": _f32(np.asarray(bq)[sl]),
            "bk": _f32(np.asarray(bk)[sl]),
            "ident16": np.eye(128, dtype=np.float16),
        })
    return in_maps


def kernel(inputs, context, Wq, bq, Wk, bk, Wv, bv, Wo, bo):
    from concourse.bass_utils import run_bass_kernel_spmd

    nc = _program()
    in_maps = make_in_maps(inputs, context, Wq, bq, Wk, bk, Wv, bv, Wo, bo)
    res = run_bass_kernel_spmd(nc, in_maps, list(range(NCORES)))
    outs = [res.results[i]["out"].astype(np.float32) for i in range(NCORES)]
    corr = (np.asarray(Wo, dtype=np.float64).T @ np.asarray(bv, dtype=np.float64)
            + np.asarray(bo, dtype=np.float64)).astype(np.float32)
    full = np.stack([
        outs[0] + outs[1] + outs[2] + outs[3],
        outs[4] + outs[5] + outs[6] + outs[7],
    ]) + corr
    return full.astype(np.float32)


# revision 28
# speedup vs baseline: 1.9281x; 1.3227x over previous
"""Cross-attention Trainium2 kernel (8 NeuronCores).

Sharding: batch (2) x head-groups (4 groups of 4 heads) = 8 shards.
Each core computes q/k/v projections for its 4 heads (256 cols of
Wq/Wk/Wv), attention for those heads, and a partial out-projection
through its 256 rows of Wo.  The host sums the 4 partial outputs per
batch and adds the bv @ Wo + bo correction, which commutes exactly
through the softmax average.

Layout strategy on-core (v2):
  - x/ctx are transposed on the HOST (xT: [d, s]) so projections
    contract d on partitions with no on-chip transposes; projections
    emit qT/kT ([dims, s], stationary W-chunks) and v natural
    (stationary xT-chunks).
  - scores are computed transposed (ST = k @ qT -> [sk, sq]) so the
    exp'd tiles feed the attention matmul directly as the stationary
    operand; ones-columns appended to v give the softmax denominator
    for free.
  - attention accumulates 4 sq-chunks into one PSUM bank so the
    softmax normalization runs as one reciprocal + one broadcast
    multiply per 4 chunks.
  - the emission order is tuned so the ACT engine (exp: the pacing
    engine at ~133us of work) starts as early as possible and is
    never starved: score batches are h-major and projection / attn /
    out-proj work is woven between them at the ACT drain rate.
"""

import numpy as np

import concourse.bass as bass
import concourse.mybir as mybir
import concourse.tile as tile
from concourse import bacc

B, SQ, SK, D, H, HS = 2, 2048, 2048, 1024, 16, 64
SCALE = HS ** -0.5
NCORES = 8
HG = 4            # heads per core
DG = HG * HS      # 256 projection cols per core

F32 = mybir.dt.float32
F16 = mybir.dt.float16


def build_program(loop_iters: int = 0):
    """Build the per-core SPMD Bass program."""
    nc = bacc.Bacc(None, target_bir_lowering=False, debug=False,
                   num_devices=NCORES)
    xT_d = nc.dram_tensor("xT", [D, SQ], F16, kind="ExternalInput")
    cT_d = nc.dram_tensor("cT", [D, SK], F16, kind="ExternalInput")
    wq_d = nc.dram_tensor("wq", [D, DG], F16, kind="ExternalInput")
    wk_d = nc.dram_tensor("wk", [D, DG], F16, kind="ExternalInput")
    wv_d = nc.dram_tensor("wv", [D, DG], F16, kind="ExternalInput")
    wo_d = nc.dram_tensor("wo", [DG, D], F16, kind="ExternalInput")
    bq_d = nc.dram_tensor("bq", [DG], F32, kind="ExternalInput")
    bk_d = nc.dram_tensor("bk", [DG], F32, kind="ExternalInput")
    i16_d = nc.dram_tensor("ident16", [128, 128], F16, kind="ExternalInput")
    out_d = nc.dram_tensor("out", [SQ, D], F16, kind="ExternalOutput")

    with tile.TileContext(nc) as tc:
        with (
            tc.tile_pool(name="const", bufs=1) as cp,
            tc.tile_pool(name="persist", bufs=1) as psb,
            tc.tile_pool(name="xw", bufs=2) as xwp,
            tc.tile_pool(name="cw", bufs=4) as cwp,
            tc.tile_pool(name="expp", bufs=48) as ep,
            tc.tile_pool(name="outp", bufs=3) as opool,
            tc.tile_pool(name="fin", bufs=4) as fpool,
            tc.tile_pool(name="pp", bufs=2, space="PSUM") as pp,
            tc.tile_pool(name="stp", bufs=2, space="PSUM") as stp,
            tc.tile_pool(name="atp", bufs=2, space="PSUM") as atp,
        ):
            import contextlib
            loop_ctx = tc.For_i(0, loop_iters, 1) if loop_iters else contextlib.nullcontext()
            loop_ctx.__enter__()
            ident16 = cp.tile([128, 128], F16, tag="ident16")
            wq_sb = cp.tile([128, 8, DG], F16, tag="wq")
            wk_sb = cp.tile([128, 8, DG], F16, tag="wk")
            wv_sb = cp.tile([128, 8, DG], F16, tag="wv")
            wo_sb = cp.tile([128, 2, D], F16, tag="wo")
            bq_sb = cp.tile([128, 2], F32, tag="bq")
            bk_sb = cp.tile([128, 2], F32, tag="bk")

            # one DMA queue, strict priority order: everything the first
            # score chunk needs (wq, x0, wk, c0, x1) goes first.
            def dma_wq():
                nc.sync.dma_start(out=wq_sb, in_=wq_d[:].rearrange("(c p) n -> p c n", p=128))

            def dma_bqk():
                nc.sync.dma_start(out=bq_sb, in_=bq_d[:].rearrange("(c p) -> p c", p=128))
                nc.sync.dma_start(out=bk_sb, in_=bk_d[:].rearrange("(c p) -> p c", p=128))

            def dma_wk():
                nc.sync.dma_start(out=wk_sb, in_=wk_d[:].rearrange("(c p) n -> p c n", p=128))

            def dma_wv():
                nc.sync.dma_start(out=wv_sb, in_=wv_d[:].rearrange("(c p) n -> p c n", p=128))

            def dma_ident():
                nc.sync.dma_start(out=ident16, in_=i16_d[:])

            def dma_wo():
                nc.sync.dma_start(out=wo_sb, in_=wo_d[:].rearrange("(c p) n -> p c n", p=128))

            xws = [None] * 4
            cws = [None] * 4

            def dma_x(w):
                xws[w] = xwp.tile([128, 8, 512], F16, tag="xw", name=f"xw{w}")
                nc.sync.dma_start(
                    out=xws[w],
                    in_=xT_d[:, w * 512:(w + 1) * 512].rearrange(
                        "(c p) s -> p c s", p=128))

            def dma_c(w):
                cws[w] = cwp.tile([128, 8, 512], F16, tag="cw", name=f"cw{w}")
                nc.sync.dma_start(
                    out=cws[w],
                    in_=cT_d[:, w * 512:(w + 1) * 512].rearrange(
                        "(c p) s -> p c s", p=128))

            # persistent activations
            qTs = [psb.tile([128, 2, 512], F16, tag=f"qT{w}", name=f"qT{w}") for w in range(4)]
            kTs = [psb.tile([128, 2, 512], F16, tag=f"kT{w}", name=f"kT{w}") for w in range(4)]
            vAs = [psb.tile([128, 4, HG, 68], F16, tag=f"vA{w}", name=f"vA{w}") for w in range(4)]
            aTw = [psb.tile([128, 2, 1024], F16, tag=f"aT{s}", name=f"aT{s}") for s in range(2)]

            def proj_qk_c(src, dst_T, bias_sb, w_sb, w, c):
                # one 128-dim chunk of qT/kT window w (c=0: heads 0/1)
                pq = pp.tile([128, 512], F32, tag="pp")
                for dc in range(8):
                    nc.tensor.matmul(
                        pq,
                        (w_sb[:, dc, c * 128:(c + 1) * 128]),
                        (src[:, dc, :]),
                        start=(dc == 0), stop=(dc == 7),
                    )
                nc.vector.tensor_scalar_add(
                    dst_T[w][:, c, :], pq, bias_sb[:, c:c + 1])

            def proj_q(w, c):
                proj_qk_c(xws[w], qTs, bq_sb, wq_sb, w, c)

            def proj_k(w, c):
                proj_qk_c(cws[w], kTs, bk_sb, wk_sb, w, c)

            def proj_v(w, s):
                # one 128-row sk chunk of v (natural layout, stationary ctxT)
                pv = atp.tile([128, 512], F32, tag="at")
                for dc in range(8):
                    nc.tensor.matmul(
                        pv[:, :DG],
                        (cws[w][:, dc, s * 128:(s + 1) * 128]),
                        (wv_sb[:, dc, :]),
                        start=(dc == 0), stop=(dc == 7),
                    )
                nc.vector.tensor_copy(
                    vAs[w][:, s, :, 0:64],
                    pv[:, :DG].rearrange("p (h e) -> p h e", e=64),
                )

            # score+exp chunk: ST[sk 128, sq 1024] for (h, sqw, skc)
            exd = {}

            def emit_st(h, sqw, skcs):
                p0 = 64 * (h % 2)
                t = h // 2
                for skc in skcs:
                    st = stp.tile([128, 1024], F32, tag="st")
                    for half in range(2):
                        nc.tensor.matmul(
                            st[:, half * 512:(half + 1) * 512],
                            (kTs[skc // 4][p0:p0 + 64, t,
                                           (skc % 4) * 128:(skc % 4 + 1) * 128]),
                            (qTs[sqw * 2 + half][p0:p0 + 64, t, :]),
                            start=True, stop=True,
                        )
                    ex = ep.tile([128, 1024], F16, tag="ex")
                    nc.scalar.activation(
                        ex, st, mybir.ActivationFunctionType.Exp,
                        scale=SCALE)
                    exd[(h, sqw, skc)] = ex

            # attention group (h, sqw, jj): 4 sq-chunks accumulated in one
            # PSUM bank -> batched normalize (1 reciprocal + 1 bcast mult).
            def attn_mm(at4, h, sqw, jj, j4, skcs):
                # one PSUM accumulation group per 128x68 slot: must run
                # start..stop consecutively (interleaving open groups on
                # one bank corrupts accumulation)
                for skc in skcs:
                    nc.tensor.matmul(
                        at4[:, j4 * 128:j4 * 128 + 68],
                        exd[(h, sqw, skc)][:, (jj * 4 + j4) * 128:
                                           (jj * 4 + j4 + 1) * 128],
                        vAs[skc // 4][:, skc % 4, h, :],
                        start=(skc == 0), stop=(skc == 15),
                    )

            def attn_fin(at4, h, sqw, jj):
                p0 = 64 * (h % 2)
                t = h // 2
                atv = at4[:].rearrange("p (j c) -> p j c", c=128)
                rc4 = fpool.tile([128, 4], F32, tag="rc")
                nc.vector.reciprocal(rc4[:, :].unsqueeze(2), atv[:, :, 64:65])
                ad4 = fpool.tile([128, 4, 64], F16, tag="ad")
                nc.vector.tensor_tensor(
                    out=ad4[:, :, :],
                    in0=atv[:, :, 0:64],
                    in1=rc4[:, :].unsqueeze(2).broadcast_to((128, 4, 64)),
                    op=mybir.AluOpType.mult,
                )
                ptw = atp.tile([128, 512], F16, tag="at")
                for j4 in range(4):
                    nc.tensor.transpose(
                        ptw[p0:p0 + 64, j4 * 128:(j4 + 1) * 128],
                        ad4[:, j4, :],
                        ident16,
                    )
                nc.vector.tensor_copy(
                    aTw[sqw][p0:p0 + 64, t, jj * 512:(jj + 1) * 512],
                    ptw[p0:p0 + 64, :])

            def emit_attn_g(h, sqw, jj):
                at4 = atp.tile([128, 512], F32, tag="at")
                for j4 in range(4):
                    attn_mm(at4, h, sqw, jj, j4, range(16))
                attn_fin(at4, h, sqw, jj)

            # out projection for a range of sq chunks (partial out: this
            # core's 256 attn cols)
            def emit_out_proj(sqcs, use_act=False):
                # tail chunks copy PSUM->SBUF on the (then idle) ACT engine
                for sqc in sqcs:
                    sqw, i = sqc // 8, sqc % 8
                    ot = opool.tile([128, D], F16, tag="ot")
                    for n2 in range(2):
                        po = pp.tile([128, 512], F32, tag="pp")
                        for kc in range(2):
                            nc.tensor.matmul(
                                po,
                                (aTw[sqw][:, kc, i * 128:(i + 1) * 128]),
                                (wo_sb[:, kc, n2 * 512:(n2 + 1) * 512]),
                                start=(kc == 0), stop=(kc == 1),
                            )
                        if use_act and n2 == 1:
                            nc.scalar.copy(ot[:, n2 * 512:(n2 + 1) * 512], po)
                        else:
                            nc.vector.tensor_copy(ot[:, n2 * 512:(n2 + 1) * 512], po)
                    nc.gpsimd.dma_start(
                        out=out_d[sqc * 128:(sqc + 1) * 128, :], in_=ot)

            # ---------------- emission schedule ----------------
            # Two DMA queues run in parallel: SP carries wq/x windows,
            # GPSIMD carries ident/wk/wv/ctx windows/wo.  The ST stream is
            # the ACT clock; all other PE work (projection c-chunks, v
            # s-chunks, attn j-groups, out-proj chunks) is woven between
            # STs so ACT is fed from ~10us on and PE never idles long.
            dma_wq()
            dma_bqk()
            dma_x(0)
            dma_x(1)
            dma_wk()
            dma_c(0)
            dma_c(1)
            dma_wv()
            dma_c(2)
            dma_c(3)
            dma_ident()
            dma_x(2)
            dma_wo()
            dma_x(3)

            for w in range(4):
                nc.vector.memset(vAs[w][:, :, :, 64:68], 1.0)

            # PE warmup: dummy matmuls on a zeroed tile so the PE is at
            # full p-state when the first real projection arrives
            warm = cp.tile([128, 512], F16, tag="warm")
            nc.vector.memset(warm[:], 0.0)
            wp = pp.tile([128, 512], F32, tag="pp", name="wp")
            for i in range(14):
                nc.tensor.matmul(wp, warm[:, 0:128], warm[:, :],
                                 start=True, stop=True)

            # lead-in: minimum projections for the first ST pair
            proj_q(0, 0)
            proj_q(1, 0)
            proj_k(0, 0)

            # phase P0: windows (0,0) and (1,0), skc-major; weave c0
            # chunks of later kT windows (gates) + remaining projections
            p0_filler = {
                1: lambda: proj_k(1, 0),
                3: lambda: proj_k(2, 0),
                5: lambda: proj_k(3, 0),
                7: lambda: proj_q(0, 1),
                9: lambda: proj_v(0, 0),
                10: lambda: proj_v(0, 1),
                11: lambda: proj_k(0, 1),
                12: lambda: proj_q(1, 1),
                13: lambda: proj_k(1, 1),
                14: lambda: proj_v(0, 2),
                15: lambda: proj_v(0, 3),
            }
            for skc in range(16):
                emit_st(0, 0, [skc])
                if skc in p0_filler:
                    p0_filler[skc]()
                emit_st(1, 0, [skc])

            # phase P1: windows (2,0) and (3,0); weave remaining kT c1
            # chunks, v windows, attn(0,0)/(1,0) groups
            p1_filler = {
                0: lambda: proj_v(1, 0),
                1: lambda: proj_v(1, 1),
                2: lambda: proj_k(2, 1),
                3: lambda: proj_v(1, 2),
                4: lambda: proj_v(1, 3),
                5: lambda: proj_k(3, 1),
                6: lambda: proj_v(2, 0),
                7: lambda: proj_v(2, 1),
                8: lambda: (proj_v(2, 2), proj_v(2, 3)),
                9: lambda: (proj_v(3, 0), proj_v(3, 1)),
                10: lambda: (proj_v(3, 2), proj_v(3, 3)),
                11: lambda: emit_attn_g(0, 0, 0),
                12: lambda: emit_attn_g(0, 0, 1),
                13: lambda: emit_attn_g(1, 0, 0),
                14: lambda: emit_attn_g(1, 0, 1),
                15: lambda: proj_q(2, 0),
            }
            for skc in range(16):
                emit_st(2, 0, [skc])
                if skc in p1_filler:
                    p1_filler[skc]()
                emit_st(3, 0, [skc])

            # phase P2: windows (0,1) and (1,1); weave qT2/3, attn(2,0)/
            # (3,0), out_proj 0-7 (ready after attn(3,0))
            proj_q(3, 0)
            p2_filler = {
                0: lambda: proj_q(2, 1),
                2: lambda: proj_q(3, 1),
                4: lambda: emit_attn_g(2, 0, 0),
                6: lambda: emit_attn_g(2, 0, 1),
                8: lambda: emit_attn_g(3, 0, 0),
                9: lambda: emit_attn_g(3, 0, 1),
                10: lambda: emit_out_proj([0]),
                11: lambda: emit_out_proj([1]),
                12: lambda: emit_out_proj([2]),
                13: lambda: emit_out_proj([3]),
                14: lambda: emit_out_proj([4]),
                15: lambda: emit_out_proj([5]),
            }
            for skc in range(16):
                emit_st(0, 1, [skc])
                if skc in p2_filler:
                    p2_filler[skc]()
                emit_st(1, 1, [skc])

            # phase P3a: window (2,1) alone; weave attn(0,1)/(1,1) and
            # out_proj 6-7 (ST cadence ~1 per ACT drain)
            p3a_filler = {
                1: lambda: emit_out_proj([6]),
                3: lambda: emit_out_proj([7]),
                5: lambda: emit_attn_g(0, 1, 0),
                7: lambda: emit_attn_g(0, 1, 1),
                9: lambda: emit_attn_g(1, 1, 0),
                11: lambda: emit_attn_g(1, 1, 1),
            }
            for skc in range(16):
                emit_st(2, 1, [skc])
                if skc in p3a_filler:
                    p3a_filler[skc]()

            # phase P3b: window (3,1) alone; attn(2,1) + out_proj 0-3
            # early, then (3,1)'s attn matmuls interleaved with lag 4 so
            # only a sliver of attention work remains after the last exp.
            p3b_filler = {
                5: lambda: emit_attn_g(2, 1, 0),
                8: lambda: emit_attn_g(2, 1, 1),
            }
            for skc in range(16):
                emit_st(3, 1, [skc])
                if skc in p3b_filler:
                    p3b_filler[skc]()

            # tail: attn(3,1) + out_proj 8-15 (ACT helps copy)
            emit_attn_g(3, 1, 0)
            emit_out_proj(range(8, 12), use_act=True)
            emit_attn_g(3, 1, 1)
            emit_out_proj(range(12, 16), use_act=True)
            loop_ctx.__exit__(None, None, None)

    nc.compile()
    return nc


_NC = None


def _program():
    global _NC
    if _NC is None:
        _NC = build_program()
    return _NC


def _f32(a):
    return np.ascontiguousarray(np.asarray(a, dtype=np.float32))


def make_in_maps(inputs, context, Wq, bq, Wk, bk, Wv, bv, Wo, bo):
    xT = [np.ascontiguousarray(np.asarray(inputs[b]).T.astype(np.float16))
          for b in range(B)]
    cT = [np.ascontiguousarray(np.asarray(context[b]).T.astype(np.float16))
          for b in range(B)]
    in_maps = []
    for core in range(NCORES):
        b, g = core // HG, core % HG
        sl = slice(DG * g, DG * (g + 1))
        in_maps.append({
            "xT": xT[b],
            "cT": cT[b],
            "wq": np.ascontiguousarray(np.asarray(Wq)[:, sl].astype(np.float16)),
            "wk": np.ascontiguousarray(np.asarray(Wk)[:, sl].astype(np.float16)),
            "wv": np.ascontiguousarray(np.asarray(Wv)[:, sl].astype(np.float16)),
            "wo": np.ascontiguousarray(np.asarray(Wo)[sl, :].astype(np.float16)),
            "bq": _f32(np.asarray(bq)[sl]),
            "bk": _f32(np.asarray(bk)[sl]),
            "ident16": np.eye(128, dtype=np.float16),
        })
    return in_maps


def kernel(inputs, context, Wq, bq, Wk, bk, Wv, bv, Wo, bo):
    from concourse.bass_utils import run_bass_kernel_spmd

    nc = _program()
    in_maps = make_in_maps(inputs, context, Wq, bq, Wk, bk, Wv, bv, Wo, bo)
    res = run_bass_kernel_spmd(nc, in_maps, list(range(NCORES)))
    outs = [res.results[i]["out"].astype(np.float32) for i in range(NCORES)]
    corr = (np.asarray(Wo, dtype=np.float64).T @ np.asarray(bv, dtype=np.float64)
            + np.asarray(bo, dtype=np.float64)).astype(np.float32)
    full = np.stack([
        outs[0] + outs[1] + outs[2] + outs[3],
        outs[4] + outs[5] + outs[6] + outs[7],
    ]) + corr
    return full.astype(np.float32)
